# revision 2
# baseline (speedup 1.0000x reference)
"""Distributed attention kernel for Trainium2 (8 NeuronCores).

Problem: B=4, N=2048, DIM=1024, HEADS=16, DIM_HEAD=64 attention with a
[1,16,2048,2048] relative-position bias, including the four linear
projections (Wq/Wk/Wv/Wo).

Sharding (Ulysses-style):
  - Stage P: token-sharded projections. Core c owns global token chunk c
    (1024 tokens = batch c//2, half c%2) and computes full-INNER qh/kh/vh
    for its tokens.
  - AllToAll x3 switches to head sharding: core c receives heads {2c,2c+1}
    for ALL 8192 tokens.
  - Stage A: attention per (batch, head) with the bias applied as
    exp(S)*exp(bias) (exp(bias^T) precomputed on host, bf16).
    S is computed transposed (S^T[j,i]) so the softmax denominator falls
    out of the AV matmul via a ones-augmented V, avoiding any
    cross-partition reduction.
  - AllToAll back to token sharding; Stage O: output projection; host
    just concatenates the 8 token slices.

All matmuls run in bf16 (fp32 matmul is 4x slower on TensorE); PSUM
accumulation is fp32. Softmax exp runs on ScalarE in fp32->bf16.
"""
import sys

sys.path.insert(0, "/opt/trn_rl_repo")

import numpy as np

import concourse.bass as bass
import concourse.bacc as bacc
import concourse.mybir as mybir
import concourse.tile as tile
from concourse import bass_utils

NCORES = 8
B, N, DIM = 4, 2048, 1024
HEADS, DH = 16, 64
INNER = HEADS * DH  # 1024
TOK = (B * N) // NCORES  # 1024 local tokens per core
HPC = HEADS // NCORES  # 2 heads per core
SCALE = DH ** -0.5

BF16 = mybir.dt.bfloat16
F32 = mybir.dt.float32
NP_BF16 = mybir.dt.np(BF16)

_EXP = mybir.ActivationFunctionType.Exp


def build_nc():
    nc = bacc.Bacc("TRN2", target_bir_lowering=False, debug=False,
                   num_devices=NCORES)

    xq = nc.dram_tensor("xq", [DIM, TOK], BF16, kind="ExternalInput").ap()
    xk = nc.dram_tensor("xk", [DIM, TOK], BF16, kind="ExternalInput").ap()
    xv = nc.dram_tensor("xv", [DIM, TOK], BF16, kind="ExternalInput").ap()
    wq = nc.dram_tensor("wq", [DIM, INNER], BF16, kind="ExternalInput").ap()
    wk = nc.dram_tensor("wk", [DIM, INNER], BF16, kind="ExternalInput").ap()
    wv = nc.dram_tensor("wv", [DIM, INNER], BF16, kind="ExternalInput").ap()
    wo = nc.dram_tensor("wo", [INNER, DIM], BF16, kind="ExternalInput").ap()
    expb = nc.dram_tensor("expb", [HPC, N, N], BF16, kind="ExternalInput").ap()
    out = nc.dram_tensor("out", [TOK, DIM], F32, kind="ExternalOutput").ap()

    rg = [list(range(NCORES))]

    with tile.TileContext(nc) as tc:
        with tc.tile_pool(name="dram", bufs=1, space="DRAM") as dram, \
             tc.tile_pool(name="wpool", bufs=10) as wpool, \
             tc.tile_pool(name="xpool", bufs=10) as xpool, \
             tc.tile_pool(name="cast", bufs=6) as cast_pool, \
             tc.tile_pool(name="psum", bufs=2, space="PSUM") as psum_pool, \
             tc.tile_pool(name="psum_s", bufs=2, space="PSUM") as psum_s_pool, \
             tc.tile_pool(name="psum_o", bufs=2, space="PSUM") as psum_o_pool, \
             tc.tile_pool(name="psum_b", bufs=1, space="PSUM") as psum_b_pool, \
             tc.tile_pool(name="qk_sb", bufs=17) as qk_pool, \
             tc.tile_pool(name="vh_sb", bufs=64) as vh_pool, \
             tc.tile_pool(name="pexp", bufs=40) as pexp_pool, \
             tc.tile_pool(name="bias", bufs=6) as bias_pool, \
             tc.tile_pool(name="small", bufs=4) as small_pool, \
             tc.tile_pool(name="ot", bufs=6) as ot_pool, \
             tc.tile_pool(name="odrain", bufs=4) as od_pool:

            # -------- collective bounce buffers (internal DRAM) --------
            qh_in = dram.tile([INNER, TOK], BF16)
            kh_in = dram.tile([INNER, TOK], BF16)
            vh_in = dram.tile([NCORES * TOK, HPC * DH], BF16)
            qh_out = dram.tile([INNER, TOK], BF16)
            kh_out = dram.tile([INNER, TOK], BF16)
            vh_out = dram.tile([NCORES * TOK, HPC * DH], BF16)
            ao_in = dram.tile([NCORES * HPC * DH, TOK], BF16)
            ao_out = dram.tile([NCORES * HPC * DH, TOK], BF16)

            # ================= Stage P: projections =================
            # qh^T / kh^T (feature-major): psum[e128, t512] = sum_cb
            #   wT[cb][:, e-slice].T @ xT[cb][:, t-slice]
            def proj_fmajor(w_dram, x_dram, dst):
                for tc_ in range(2):
                    x_tiles = []
                    for cb in range(8):
                        xt = xpool.tile([128, 512], BF16, tag="x_sb")
                        nc.sync.dma_start(
                            xt[:], x_dram[cb * 128:(cb + 1) * 128,
                                          tc_ * 512:(tc_ + 1) * 512])
                        x_tiles.append(xt)
                    w_tiles = []
                    for cb in range(8):
                        wt = wpool.tile([128, INNER], BF16, tag="w_sb")
                        nc.sync.dma_start(
                            wt[:], w_dram[cb * 128:(cb + 1) * 128, :])
                        w_tiles.append(wt)
                    for eb in range(8):
                        ps = psum_pool.tile([128, 512], F32)
                        for cb in range(8):
                            nc.tensor.matmul(
                                ps[:], w_tiles[cb][:, eb * 128:(eb + 1) * 128],
                                x_tiles[cb][:], start=(cb == 0), stop=(cb == 7))
                        sb = cast_pool.tile([128, 512], BF16, tag="pdrain")
                        nc.scalar.copy(sb[:], ps[:])
                        nc.sync.dma_start(
                            dst[eb * 128:(eb + 1) * 128,
                                tc_ * 512:(tc_ + 1) * 512], sb[:])

            # vh (token-major): psum[t128, e512] = sum_cb
            #   xT[cb][:, t-slice].T @ wT[cb][:, e-slice]
            def proj_tmajor(w_dram, x_dram, dst):
                for ec in range(2):
                    x_tiles = []
                    for cb in range(8):
                        xt = xpool.tile([128, TOK], BF16, tag="x_sb")
                        nc.sync.dma_start(
                            xt[:], x_dram[cb * 128:(cb + 1) * 128, :])
                        x_tiles.append(xt)
                    w_tiles = []
                    for cb in range(8):
                        wt = wpool.tile([128, 512], BF16, tag="w_sb")
                        nc.sync.dma_start(
                            wt[:], w_dram[cb * 128:(cb + 1) * 128,
                                          ec * 512:(ec + 1) * 512])
                        w_tiles.append(wt)
                    for tb in range(8):
                        ps = psum_pool.tile([128, 512], F32)
                        for cb in range(8):
                            nc.tensor.matmul(
                                ps[:],
                                x_tiles[cb][:, tb * 128:(tb + 1) * 128],
                                w_tiles[cb][:], start=(cb == 0), stop=(cb == 7))
                        sb = cast_pool.tile([128, 512], BF16, tag="pdrain")
                        nc.scalar.copy(sb[:], ps[:])
                        # scatter the 4 e-col blocks into shard-major layout
                        for sj in range(4):
                            shard = ec * 4 + sj
                            nc.sync.dma_start(
                                dst[shard * TOK + tb * 128:
                                    shard * TOK + (tb + 1) * 128, :],
                                sb[:, sj * 128:(sj + 1) * 128])

            proj_fmajor(wq, xq, qh_in)
            nc.gpsimd.collective_compute(
                "AllToAll", mybir.AluOpType.bypass, replica_groups=rg,
                ins=[qh_in.opt()], outs=[qh_out.opt()])
            proj_fmajor(wk, xk, kh_in)
            nc.gpsimd.collective_compute(
                "AllToAll", mybir.AluOpType.bypass, replica_groups=rg,
                ins=[kh_in.opt()], outs=[kh_out.opt()])
            proj_tmajor(wv, xv, vh_in)
            nc.gpsimd.collective_compute(
                "AllToAll", mybir.AluOpType.bypass, replica_groups=rg,
                ins=[vh_in.opt()], outs=[vh_out.opt()])

            # ============ Stage A: attention (2 heads, 4 batches) ============
            # SBUF-resident qh/kh (feature-major, per global token chunk)
            qh_sb, kh_sb = [], []
            for t in range(NCORES):
                qt = qk_pool.tile([128, TOK], BF16, tag="big")
                nc.sync.dma_start(qt[:], qh_out[t * 128:(t + 1) * 128, :])
                qh_sb.append(qt)
                kt = qk_pool.tile([128, TOK], BF16, tag="big")
                nc.sync.dma_start(kt[:], kh_out[t * 128:(t + 1) * 128, :])
                kh_sb.append(kt)
            # vh (token-major) augmented with a ones column per head:
            # [h0 d64 | 1 | h1 d64 | 1]
            vh_aug = []
            for g in range(64):
                vt = vh_pool.tile([128, 2 * (DH + 1)], BF16, tag="vh_aug")
                nc.sync.dma_start(vt[:, 0:DH],
                                  vh_out[g * 128:(g + 1) * 128, 0:DH])
                nc.sync.dma_start(vt[:, DH + 1:2 * DH + 1],
                                  vh_out[g * 128:(g + 1) * 128, DH:2 * DH])
                nc.vector.memset(vt[:, DH:DH + 1], 1.0)
                nc.vector.memset(vt[:, 2 * DH + 1:2 * DH + 2], 1.0)
                vh_aug.append(vt)
            ones_sb = small_pool.tile([1, 64], BF16, tag="ones")
            nc.vector.memset(ones_sb[:], 1.0)

            for b in range(B):
                for ic in range(4):  # query chunk of 512 within batch b
                    i_t = 2 * b + ic // 2          # global token chunk
                    i_off = (ic % 2) * 512
                    p_tiles = {0: [], 1: []}
                    for jb in range(16):  # key block of 128 within batch b
                        t_j = 2 * b + jb // 8
                        jj = jb % 8
                        for h in range(HPC):
                            ps = psum_s_pool.tile([128, 512], F32)
                            nc.tensor.matmul(
                                ps[:],
                                kh_sb[t_j][h * DH:(h + 1) * DH,
                                           jj * 128:(jj + 1) * 128],
                                qh_sb[i_t][h * DH:(h + 1) * DH,
                                           i_off:i_off + 512],
                                start=True, stop=True)
                            eb_t = bias_pool.tile([128, 512], BF16, tag="eb")
                            nc.sync.dma_start(
                                eb_t[:],
                                expb[h, jb * 128:(jb + 1) * 128,
                                     (ic * 512):(ic * 512 + 512)])
                            es = pexp_pool.tile([128, 512], BF16, tag="es")
                            nc.scalar.activation(es[:], ps[:], _EXP)
                            nc.vector.tensor_mul(es[:], es[:], eb_t[:])
                            p_tiles[h].append(es)
                    for h in range(HPC):
                        po = psum_o_pool.tile([DH + 1, 512], F32)
                        for jb in range(16):
                            g = (2 * b + jb // 8) * 8 + jb % 8
                            nc.tensor.matmul(
                                po[:],
                                vh_aug[g][:, h * (DH + 1):(h + 1) * (DH + 1)],
                                p_tiles[h][jb][:],
                                start=(jb == 0), stop=(jb == 15))
                        den = small_pool.tile([1, 512], F32, tag="sc32")
                        nc.scalar.copy(den[:], po[DH:DH + 1, :])
                        rec = small_pool.tile([1, 512], F32, tag="sc32")
                        nc.vector.reciprocal(rec[:], den[:])
                        recb = small_pool.tile([1, 512], BF16, tag="recb")
                        nc.vector.tensor_copy(recb[:], rec[:])
                        pb = psum_b_pool.tile([64, 512], F32)
                        nc.tensor.matmul(pb[:], ones_sb[:], recb[:],
                                         start=True, stop=True)
                        rb = cast_pool.tile([64, 512], BF16, tag="rb")
                        nc.scalar.copy(rb[:], pb[:])
                        ot = ot_pool.tile([64, 512], BF16, tag="ot")
                        nc.vector.tensor_mul(ot[:], po[0:DH, :], rb[:])
                        nc.sync.dma_start(
                            ao_in[i_t * 128 + h * DH:i_t * 128 + (h + 1) * DH,
                                  i_off:i_off + 512], ot[:])

            nc.gpsimd.collective_compute(
                "AllToAll", mybir.AluOpType.bypass, replica_groups=rg,
                ins=[ao_in.opt()], outs=[ao_out.opt()])

            # ================= Stage O: output projection =================
            ao_sb = []
            for eb in range(8):
                at = qk_pool.tile([128, TOK], BF16, tag="big")
                nc.sync.dma_start(at[:], ao_out[eb * 128:(eb + 1) * 128, :])
                ao_sb.append(at)
            wo_sb = []
            for eb in range(8):
                wt = wpool.tile([128, DIM], BF16, tag="w_sb")
                nc.sync.dma_start(wt[:], wo[eb * 128:(eb + 1) * 128, :])
                wo_sb.append(wt)
            for tb in range(8):
                for oc in range(2):
                    ps = psum_pool.tile([128, 512], F32)
                    for eb in range(8):
                        nc.tensor.matmul(
                            ps[:], ao_sb[eb][:, tb * 128:(tb + 1) * 128],
                            wo_sb[eb][:, oc * 512:(oc + 1) * 512],
                            start=(eb == 0), stop=(eb == 7))
                    ob = od_pool.tile([128, 512], F32, tag="odrain")
                    nc.scalar.copy(ob[:], ps[:])
                    nc.sync.dma_start(
                        out[tb * 128:(tb + 1) * 128,
                            oc * 512:(oc + 1) * 512], ob[:])

    nc.compile()
    return nc


def make_in_maps(q, k, v, rel_pos_bias, Wq, Wk, Wv, Wo):
    """Host-side sharding: transposes, bf16 casts, exp(bias)."""
    q = np.asarray(q, np.float32)
    k = np.asarray(k, np.float32)
    v = np.asarray(v, np.float32)
    bias = np.asarray(rel_pos_bias, np.float32)
    wq_t = np.ascontiguousarray((np.asarray(Wq) * SCALE).T).astype(NP_BF16)
    wk_t = np.ascontiguousarray(np.asarray(Wk).T).astype(NP_BF16)
    wv_t = np.ascontiguousarray(np.asarray(Wv).T).astype(NP_BF16)
    wo_t = np.ascontiguousarray(np.asarray(Wo).T).astype(NP_BF16)
    in_maps = []
    for c in range(NCORES):
        b, half = c // 2, c % 2
        ts = slice(half * TOK, (half + 1) * TOK)
        expb = np.exp(bias[0, HPC * c:HPC * (c + 1)].transpose(0, 2, 1))
        in_maps.append({
            "xq": np.ascontiguousarray(q[b, ts, :].T).astype(NP_BF16),
            "xk": np.ascontiguousarray(k[b, ts, :].T).astype(NP_BF16),
            "xv": np.ascontiguousarray(v[b, ts, :].T).astype(NP_BF16),
            "wq": wq_t, "wk": wk_t, "wv": wv_t, "wo": wo_t,
            "expb": expb.astype(NP_BF16),
        })
    return in_maps


_NC_CACHE = None


def kernel(q, k, v, rel_pos_bias, Wq, Wk, Wv, Wo):
    global _NC_CACHE
    if _NC_CACHE is None:
        _NC_CACHE = build_nc()
    nc = _NC_CACHE
    in_maps = make_in_maps(q, k, v, rel_pos_bias, Wq, Wk, Wv, Wo)
    res = bass_utils.run_bass_kernel_spmd(nc, in_maps,
                                          core_ids=list(range(NCORES)))
    out = np.empty((B, N, DIM), np.float32)
    for c in range(NCORES):
        b, half = c // 2, c % 2
        out[b, half * TOK:(half + 1) * TOK, :] = res.results[c]["out"]
    return out


# revision 31
# speedup vs baseline: 152.3494x; 152.3494x over previous
"""Distributed attention kernel for Trainium2 (8 NeuronCores).

Problem: B=4, N=2048, DIM=1024, HEADS=16, DIM_HEAD=64 attention with a
[1,16,2048,2048] relative-position bias, including the four linear
projections (Wq/Wk/Wv/Wo).

Sharding (Ulysses-style):
  - Stage P: token-sharded projections. Core c owns global token chunk c
    (1024 tokens = batch c//2, half c%2) and computes full-INNER qh/kh/vh
    for its tokens.
  - AllToAll x3 switches to head sharding: core c receives heads {2c,2c+1}
    for ALL 8192 tokens.
  - Stage A: attention per (batch, head) with the bias applied as
    exp(S)*exp(bias) (exp(bias^T) precomputed on host, bf16).
    S is computed transposed (S^T[j,i]) so the softmax denominator falls
    out of the AV matmul via a ones-augmented V, avoiding any
    cross-partition reduction.
  - AllToAll back to token sharding; Stage O: output projection; host
    just concatenates the 8 token slices.

All matmuls run in bf16 (fp32 matmul is 4x slower on TensorE); PSUM
accumulation is fp32. Softmax exp runs on ScalarE in fp32->bf16.

DMA discipline (the first-order cost on TRN2 after compute): every
dma_start holds the issuing sequencer ~650ns and the HWDGE ~625ns, so
the kernel uses few, large DMAs: bias tiles are loaded once per query
chunk (reused across all 4 batches) from a host-packed layout, x/w tiles
are hoisted out of accumulation loops, and scatter/gather DMAs use
multi-dim access patterns instead of multiple small copies. Bias loads
go through gpsimd (SWDGE) to stay off the HWDGE path.
"""
import sys

sys.path.insert(0, "/opt/trn_rl_repo")

import numpy as np

import concourse.bass as bass
import concourse.bacc as bacc
import concourse.mybir as mybir
import concourse.tile as tile
from concourse import bass_utils

NCORES = 8
B, N, DIM = 4, 2048, 1024
HEADS, DH = 16, 64
INNER = HEADS * DH  # 1024
TOK = (B * N) // NCORES  # 1024 local tokens per core
HPC = HEADS // NCORES  # 2 heads per core
SCALE = DH ** -0.5

BF16 = mybir.dt.bfloat16
F32 = mybir.dt.float32
NP_BF16 = mybir.dt.np(BF16)

_EXP = mybir.ActivationFunctionType.Exp


def build_nc(mock_collectives=False):
    nc = bacc.Bacc("TRN2", target_bir_lowering=False, debug=False,
                   num_devices=NCORES)

    xq = nc.dram_tensor("xq", [DIM, TOK], BF16, kind="ExternalInput").ap()
    xk = nc.dram_tensor("xk", [DIM, TOK], BF16, kind="ExternalInput").ap()
    xv = nc.dram_tensor("xv", [DIM, TOK], BF16, kind="ExternalInput").ap()
    wq = nc.dram_tensor("wq", [DIM, INNER], BF16, kind="ExternalInput").ap()
    wk = nc.dram_tensor("wk", [DIM, INNER], BF16, kind="ExternalInput").ap()
    wv = nc.dram_tensor("wv", [DIM, INNER], BF16, kind="ExternalInput").ap()
    wo = nc.dram_tensor("wo", [INNER, DIM], BF16, kind="ExternalInput").ap()
    # packed: expb[jb*128+j, ic*1024 + h*512 + iw] = exp(bias[h, i, j])
    expb = nc.dram_tensor("expb", [N, 2 * N], BF16, kind="ExternalInput").ap()
    out = nc.dram_tensor("out", [TOK, DIM], F32, kind="ExternalOutput").ap()

    rg = [list(range(NCORES))]

    def a2a(in_t, out_t):
        if mock_collectives:
            nc.sync.dma_start(out_t[:], in_t[:])
        else:
            nc.gpsimd.collective_compute(
                "AllToAll", mybir.AluOpType.bypass, replica_groups=rg,
                ins=[in_t.opt()], outs=[out_t.opt()])

    with tile.TileContext(nc) as tc:
        with tc.tile_pool(name="dram", bufs=1, space="DRAM") as dram, \
             tc.tile_pool(name="wpool", bufs=16) as wpool, \
             tc.tile_pool(name="xpool", bufs=16) as xpool, \
             tc.tile_pool(name="cast", bufs=4) as cast_pool, \
             tc.tile_pool(name="psum", bufs=2, space="PSUM") as psum_pool, \
             tc.tile_pool(name="psum_s", bufs=2, space="PSUM") as psum_s_pool, \
             tc.tile_pool(name="psum_o", bufs=2, space="PSUM") as psum_o_pool, \
             tc.tile_pool(name="qk_sb", bufs=16) as qk_pool, \
             tc.tile_pool(name="vh_sb", bufs=64) as vh_pool, \
             tc.tile_pool(name="pexp", bufs=18) as pexp_pool, \
             tc.tile_pool(name="bias", bufs=18) as bias_pool, \
             tc.tile_pool(name="small", bufs=1) as small_pool, \
             tc.tile_pool(name="ot", bufs=3) as ot_pool, \
             tc.tile_pool(name="odrain", bufs=2) as od_pool:

            # -------- collective bounce buffers (internal DRAM) --------
            qk_in = dram.tile([2 * INNER, TOK], BF16)
            qk_out = dram.tile([2 * INNER, TOK], BF16)
            vh_in = dram.tile([NCORES * TOK, HPC * DH], BF16)
            vh_out = dram.tile([NCORES * TOK, HPC * DH], BF16)
            ao_in = dram.tile([NCORES * (HPC * DH + HPC), TOK], BF16)
            ao_out = dram.tile([NCORES * (HPC * DH + HPC), TOK], BF16)

            # ================= Stage P: projections =================
            # qh^T / kh^T (feature-major): psum[e128, t512] = sum_cb
            #   wT[cb][:, e-slice].T @ xT[cb][:, t-slice]
            def proj_fmajor(w_dram, x_dram, dst, row_off):
                x_tiles, w_tiles = [], []
                for cb in range(8):
                    xt = xpool.tile([128, TOK], BF16, tag="x_sb")
                    nc.sync.dma_start(xt[:], x_dram[cb * 128:(cb + 1) * 128, :])
                    x_tiles.append(xt)
                    wt = wpool.tile([128, INNER], BF16, tag="w_sb")
                    nc.gpsimd.dma_start(wt[:],
                                        w_dram[cb * 128:(cb + 1) * 128, :])
                    w_tiles.append(wt)
                for tc_ in range(2):
                    for eb in range(8):
                        ps = psum_pool.tile([128, 512], F32)
                        for cb in range(8):
                            nc.tensor.matmul(
                                ps[:], w_tiles[cb][:, eb * 128:(eb + 1) * 128],
                                x_tiles[cb][:, tc_ * 512:(tc_ + 1) * 512],
                                start=(cb == 0), stop=(cb == 7))
                        sb = cast_pool.tile([128, 512], BF16, tag="pdrain")
                        nc.vector.tensor_copy(sb[:], ps[:])
                        row = eb * 256 + row_off  # shard eb: [qh | kh]
                        nc.sync.dma_start(
                            dst[row:row + 128,
                                tc_ * 512:(tc_ + 1) * 512], sb[:])

            # vh (token-major): psum[t128, e512] = sum_cb
            #   xT[cb][:, t-slice].T @ wT[cb][:, e-slice]
            def proj_tmajor(w_dram, x_dram, dst):
                x_tiles, w_tiles = [], []
                for cb in range(8):
                    xt = xpool.tile([128, TOK], BF16, tag="x_sb")
                    nc.sync.dma_start(xt[:], x_dram[cb * 128:(cb + 1) * 128, :])
                    x_tiles.append(xt)
                    wt = wpool.tile([128, INNER], BF16, tag="w_sb")
                    nc.gpsimd.dma_start(wt[:],
                                        w_dram[cb * 128:(cb + 1) * 128, :])
                    w_tiles.append(wt)
                for ec in range(2):
                    for tb in range(8):
                        ps = psum_pool.tile([128, 512], F32)
                        for cb in range(8):
                            nc.tensor.matmul(
                                ps[:],
                                x_tiles[cb][:, tb * 128:(tb + 1) * 128],
                                w_tiles[cb][:, ec * 512:(ec + 1) * 512],
                                start=(cb == 0), stop=(cb == 7))
                        sb = cast_pool.tile([128, 512], BF16, tag="pdrain")
                        nc.vector.tensor_copy(sb[:], ps[:])
                        # sb[t, sj*128 + c] -> dst[(ec*4+sj)*TOK + tb*128 + t, c]
                        # one scatter DMA: dst AP dims (t, sj, c)
                        dst_ap = (dst[:]
                                  .rearrange("(s t) c -> s t c", s=8)
                                  [ec * 4:(ec + 1) * 4,
                                   tb * 128:(tb + 1) * 128, :]
                                  .transpose([1, 0, 2]))
                        src_ap = sb[:].rearrange("t (s c) -> t s c", s=4)
                        nc.sync.dma_start(dst_ap, src_ap)

            proj_fmajor(wq, xq, qk_in, 0)
            proj_fmajor(wk, xk, qk_in, 128)
            a2a(qk_in, qk_out)

            # ============ Stage A: attention (2 heads, 4 batches) ============
            # SBUF-resident qh/kh (feature-major, per global token chunk)
            qh_sb, kh_sb = [], []
            for t in range(NCORES):
                qt = qk_pool.tile([128, TOK], BF16, tag="big")
                nc.scalar.dma_start(qt[:], qk_out[t * 256:t * 256 + 128, :])
                qh_sb.append(qt)
                kt = qk_pool.tile([128, TOK], BF16, tag="big")
                nc.scalar.dma_start(kt[:],
                                    qk_out[t * 256 + 128:(t + 1) * 256, :])
                kh_sb.append(kt)

            def emit_bias(ic):
                tiles = []
                for jb in range(16):
                    ebt = bias_pool.tile([128, 2 * 512], BF16, tag="eb")
                    nc.gpsimd.dma_start(
                        ebt[:], expb[jb * 128:(jb + 1) * 128,
                                     ic * 1024:(ic + 1) * 1024])
                    tiles.append(ebt)
                return tiles

            def emit_S(ic, b, jb, eb_tiles):
                # h0 | h1 packed along the free axis (matches the bias
                # tile layout) -> one exp + one mul per jb
                i_t = 2 * b + ic // 2
                i_off = (ic % 2) * 512
                t_j = 2 * b + jb // 8
                jj = jb % 8
                ps = psum_s_pool.tile([128, 1024], F32)
                for h in range(HPC):
                    nc.tensor.matmul(
                        ps[:, h * 512:(h + 1) * 512],
                        kh_sb[t_j][h * DH:(h + 1) * DH,
                                   jj * 128:(jj + 1) * 128],
                        qh_sb[i_t][h * DH:(h + 1) * DH, i_off:i_off + 512],
                        start=True, stop=True)
                es = pexp_pool.tile([128, 1024], BF16, tag="es")
                nc.scalar.activation(es[:], ps[:], _EXP)
                nc.vector.tensor_mul(es[:], es[:], eb_tiles[jb][:])
                return es

            def emit_AV_step(st, jb):
                ic, b, p_tiles, po = st
                g = (2 * b + jb // 8) * 8 + jb % 8
                for h in range(HPC):
                    nc.tensor.matmul(
                        po[h][:],
                        vh_aug[g][:, h * (DH + 1):(h + 1) * (DH + 1)],
                        p_tiles[jb][:, h * 512:(h + 1) * 512],
                        start=(jb == 0), stop=(jb == 15))

            def finish_AV(st):
                # unnormalized out^T + denominators; normalization is
                # deferred past the A2A into stage O
                ic, b, p_tiles, po = st
                i_t = 2 * b + ic // 2
                i_off = (ic % 2) * 512
                ot = ot_pool.tile([128, 512], BF16, tag="ot")
                otd = ot_pool.tile([1, 1024], BF16, tag="otd")
                for h in range(HPC):
                    nc.vector.tensor_copy(ot[h * DH:(h + 1) * DH, :],
                                          po[h][0:DH, :])
                    nc.vector.tensor_copy(otd[0:1, h * 512:(h + 1) * 512],
                                          po[h][DH:DH + 1, :])
                nc.sync.dma_start(
                    ao_in[i_t * 130:i_t * 130 + 128, i_off:i_off + 512],
                    ot[:])
                nc.sync.dma_start(
                    ao_in[i_t * 130 + 128:i_t * 130 + 130,
                          i_off:i_off + 512],
                    otd[:].rearrange("p (r c) -> p r c", r=2))

            seq = [(ic, b) for ic in range(4) for b in range(B)]
            # iteration 0's S-phase is emitted BEFORE the v-projection so
            # ScalarE starts exp while PE still runs projections
            eb_by_ic = {0: emit_bias(0)}
            p0 = [emit_S(0, 0, jb, eb_by_ic[0]) for jb in range(16)]
            prev = (0, 0, p0, None)

            proj_tmajor(wv, xv, vh_in)
            a2a(vh_in, vh_out)
            # vh (token-major) augmented with a ones column per head:
            # [h0 d64 | 1 | h1 d64 | 1]; one strided DMA + 2 memsets per block
            vh_aug = []
            for g in range(64):
                vt = vh_pool.tile([128, 2 * (DH + 1)], BF16, tag="vh_aug")
                dst_ap = vt[:].rearrange("t (s c) -> t s c", c=DH + 1)[:, :, 0:DH]
                src_ap = (vh_out[g * 128:(g + 1) * 128, :]
                          .rearrange("t (s c) -> t s c", c=DH))
                nc.scalar.dma_start(dst_ap, src_ap)
                nc.vector.memset(vt[:, DH:DH + 1], 1.0)
                nc.vector.memset(vt[:, 2 * DH + 1:2 * DH + 2], 1.0)
                vh_aug.append(vt)

            # software pipeline: S(n) interleaved with AV(n-1) at jb grain
            for n in range(1, len(seq) + 1):
                po_list = []
                for _h in range(HPC):
                    po_t = psum_o_pool.tile([DH + 1, 512], F32, tag="po")
                    po_list.append(po_t)
                prev = (prev[0], prev[1], prev[2], po_list)
                if n < len(seq):
                    ic, b = seq[n]
                    if ic not in eb_by_ic and b == 2:
                        pass  # prefetch trigger handled below
                    if ic not in eb_by_ic:
                        eb_by_ic[ic] = emit_bias(ic)
                    p_cur = []
                    if n == 1:
                        # vh A2A may still be in flight: keep AV(0,0) off the
                        # PE queue until a full S-phase covers the latency
                        for jb in range(16):
                            p_cur.append(emit_S(ic, b, jb, eb_by_ic[ic]))
                        for jb in range(16):
                            emit_AV_step(prev, jb)
                    else:
                        for jb in range(16):
                            p_cur.append(emit_S(ic, b, jb, eb_by_ic[ic]))
                            emit_AV_step(prev, jb)
                    finish_AV(prev)
                    prev = (ic, b, p_cur, None)
                else:
                    for jb in range(16):
                        emit_AV_step(prev, jb)
                    finish_AV(prev)

            a2a(ao_in, ao_out)

            # ================= Stage O: output projection =================
            # denominators: one [2, TOK] tile per source core (2 heads)
            rcp_tiles = []
            for t in range(8):
                dnt = cast_pool.tile([2, TOK], BF16, tag="pdrain")
                nc.scalar.dma_start(
                    dnt[:], ao_out[t * 130 + 128:t * 130 + 130, :])
                with nc.allow_low_precision(reason="softmax denom recip"):
                    nc.vector.reciprocal(dnt[:], dnt[:])
                rcp_tiles.append(dnt)
            # head-selector for partition-broadcast of recip rows: [2, 128]
            esel_np = np.zeros((2, 128), NP_BF16)
            esel_np[0, 0:DH] = 1.0
            esel_np[1, DH:2 * DH] = 1.0
            esel_dram = nc.inline_tensor(esel_np, name="esel_const")
            e_sel = small_pool.tile([2, 128], BF16, tag="esel")
            nc.scalar.dma_start(e_sel[:], esel_dram.ap()[:])
            ao_sb, wo_sb = [], []
            for eb in range(8):
                at = qk_pool.tile([128, TOK], BF16, tag="big")
                nc.scalar.dma_start(at[:],
                                    ao_out[eb * 130:eb * 130 + 128, :])
                ao_sb.append(at)
                wt = wpool.tile([128, DIM], BF16, tag="w_sb")
                nc.sync.dma_start(wt[:], wo[eb * 128:(eb + 1) * 128, :])
                wo_sb.append(wt)
            # normalize: aon[e, t] = ao[e, t] * recip[head(e), t]
            aon_sb = []
            for eb in range(8):
                pn = psum_s_pool.tile([128, 1024], F32, tag="ps")
                for oc in range(2):
                    nc.tensor.matmul(
                        pn[:, oc * 512:(oc + 1) * 512], e_sel[:],
                        rcp_tiles[eb][:, oc * 512:(oc + 1) * 512],
                        start=True, stop=True)
                an = pexp_pool.tile([128, 1024], BF16, tag="es")
                nc.vector.tensor_mul(an[:], ao_sb[eb][:], pn[:])
                aon_sb.append(an)
            for tb in range(8):
                for oc in range(2):
                    ps = psum_pool.tile([128, 512], F32)
                    for eb in range(8):
                        nc.tensor.matmul(
                            ps[:], aon_sb[eb][:, tb * 128:(tb + 1) * 128],
                            wo_sb[eb][:, oc * 512:(oc + 1) * 512],
                            start=(eb == 0), stop=(eb == 7))
                    ob = od_pool.tile([128, 512], F32, tag="odrain")
                    nc.vector.tensor_copy(ob[:], ps[:])
                    nc.sync.dma_start(
                        out[tb * 128:(tb + 1) * 128,
                            oc * 512:(oc + 1) * 512], ob[:])

    nc.compile()
    return nc


def make_in_maps(q, k, v, rel_pos_bias, Wq, Wk, Wv, Wo):
    """Host-side sharding: transposes, bf16 casts, exp(bias) packing."""
    q = np.asarray(q, np.float32)
    k = np.asarray(k, np.float32)
    v = np.asarray(v, np.float32)
    bias = np.asarray(rel_pos_bias, np.float32)
    wq_t = np.ascontiguousarray((np.asarray(Wq) * SCALE).T).astype(NP_BF16)
    wk_t = np.ascontiguousarray(np.asarray(Wk).T).astype(NP_BF16)
    wv_t = np.ascontiguousarray(np.asarray(Wv).T).astype(NP_BF16)
    wo_t = np.ascontiguousarray(np.asarray(Wo).T).astype(NP_BF16)
    in_maps = []
    for c in range(NCORES):
        b, half = c // 2, c % 2
        ts = slice(half * TOK, (half + 1) * TOK)
        # [h, i, j] -> exp -> packed [j, (ic, h, iw)] with i = ic*512 + iw
        eb = np.exp(bias[0, HPC * c:HPC * (c + 1)])          # [h, i, j]
        eb = eb.transpose(2, 1, 0)                            # [j, i, h]
        eb = eb.reshape(N, 4, 512, 2).transpose(0, 1, 3, 2)   # [j, ic, h, iw]
        in_maps.append({
            "xq": np.ascontiguousarray(q[b, ts, :].T).astype(NP_BF16),
            "xk": np.ascontiguousarray(k[b, ts, :].T).astype(NP_BF16),
            "xv": np.ascontiguousarray(v[b, ts, :].T).astype(NP_BF16),
            "wq": wq_t, "wk": wk_t, "wv": wv_t, "wo": wo_t,
            "expb": np.ascontiguousarray(eb.reshape(N, 2 * N)).astype(NP_BF16),
        })
    return in_maps


_NC_CACHE = None


def kernel(q, k, v, rel_pos_bias, Wq, Wk, Wv, Wo):
    global _NC_CACHE
    if _NC_CACHE is None:
        _NC_CACHE = build_nc()
    nc = _NC_CACHE
    in_maps = make_in_maps(q, k, v, rel_pos_bias, Wq, Wk, Wv, Wo)
    res = bass_utils.run_bass_kernel_spmd(nc, in_maps,
                                          core_ids=list(range(NCORES)))
    out = np.empty((B, N, DIM), np.float32)
    for c in range(NCORES):
        b, half = c // 2, c % 2
        out[b, half * TOK:(half + 1) * TOK, :] = res.results[c]["out"]
    return out
